# revision 22
# baseline (speedup 1.0000x reference)
"""Segment-mean-of-means kernel for Trainium2 (8 NeuronCores, SPMD).

Problem: out = mean_s( segment_sum(x)[s] / max(count_s, 1) ) over 65536
segments of a [4M, 64] fp32 tensor with *sorted* segment ids.

Reformulation: out[f] = (1/N0) * sum_i w_i * x_i[f] with w_i =
1/count_{seg(i)} -- a pure streaming weighted row-reduction.  The device
is HBM-bound, so x is streamed as fp8 E3M4 (host-side cast; 4 mantissa
bits -> 1.7e-2 max-rel on the target data vs the 2e-2 budget) with an
optional fp16 tail (P8 dial).  Weights stay fp16 (mixed-dtype matmul).

Device structure per core:
  - prelude: a few thin groups (32 partitions x R8 slots) so the first
    matmul only waits on a 256KB DMA instead of 1MB+w.
  - main fp8 groups (128 x R8): most go through PE matmuls accumulating
    into one PSUM bank; a few are offloaded to the otherwise-idle Vector
    engine (broadcast-multiply into a tmp tile, then a strided reduce
    into a per-group fp32 slab) to shorten the PE stream.
  - fp16 groups + a partial remainder group close out the rows.
Host sums the PSUM diagonal blocks + the DVE slab partitions, then
divides by N0.

Group layout: row j of a group lives at (partition k = j//R, slot t =
j%R); a partition's slice of a group is R*64 contiguous bytes in DRAM
(8KB at R8=128 fp8), one DMA descriptor.  Each PE group is reduced by
R/8 matmuls  lhsT = w[:, off+g*R+8j : +8] (fp16), rhs = x_sb[:,
8j*64:(8j+8)*64] -> psum[8, 512], whose diagonal 64-blocks accumulate
the weighted sums (off-diagonal blocks are garbage ignored on the host).
"""

import os

import ml_dtypes
import numpy as np

import concourse.bass as bass
import concourse.mybir as mybir
from concourse import bacc
from concourse.bass_utils import run_bass_kernel_spmd
from concourse.tile import TileContext


def _harden_trace_path():
    """If a caller enables tracing (e.g. BASS_TRACE=1), run_bass_kernel_spmd
    imports antenv.axon_hooks, which this image lacks -- that would crash the
    run.  Provide the hook via trn_boot's ctypes shim (or a None hook, which
    bass_utils degrades on gracefully), and make the artifact upload failure
    non-fatal (zero-egress sandbox)."""
    import sys
    import types

    try:
        import antenv.axon_hooks  # noqa: F401  # already provided: nothing to do
        return
    except ImportError:
        pass
    hook = None
    try:
        import trn_agent_boot.trn_boot as tb

        hook = tb._ntff_profile_via_ctypes("/opt/axon/libaxon_pjrt.so")
    except Exception:
        pass
    mod = types.ModuleType("antenv.axon_hooks")
    mod.get_axon_ntff_profile_hook = lambda: hook
    sys.modules["antenv.axon_hooks"] = mod

    import concourse.bass_utils as bu

    _orig_upload = bu.upload_artifacts

    def _safe_upload(tmpdir):
        try:
            return _orig_upload(tmpdir)
        except Exception:
            return tmpdir

    bu.upload_artifacts = _safe_upload


_harden_trace_path()

F = 64  # features
NC = 8  # cores
M = 8  # matmul M dim (psum partitions); 8*F = 512 = one PSUM bank
FP8_NP = ml_dtypes.float8_e3m4
FP8_BIR = mybir.dt.float8e3

P8 = float(os.environ.get("KERNEL_P8", "1.0"))  # fraction of rows in e3m4
R8 = int(os.environ.get("KERNEL_R8", "128"))  # fp8 rows/partition/group (8KB runs)
R16 = 64  # fp16 rows/partition/group (8KB runs)
PREP = 64  # prelude partitions (fewer descriptors -> faster first DMA)
G8_ROWS = 128 * R8  # rows per fp8 main group
GP_ROWS = PREP * R8  # rows per prelude group
G16_ROWS = 128 * R16  # rows per fp16 group
N_PRE = int(os.environ.get("KERNEL_PRE", "2"))  # thin prelude groups
N_DVE = int(os.environ.get("KERNEL_DVE", "5"))  # main fp8 groups on Vector engine
N_WARM = int(os.environ.get("KERNEL_WARM", "5"))  # PE warm-up dummy matmuls
XB8 = int(os.environ.get("KERNEL_XB8", "9"))  # fp8 x tile buffering depth
# DVE x tiles: one buffer per offloaded group, so a DVE tile's dma_start
# never waits on the slow Vector consumer to free a buffer (such a wait
# head-of-line-blocks the whole in-order DMA issue queue).
XBD = int(os.environ.get("KERNEL_XBD", "0")) or None
XB16 = int(os.environ.get("KERNEL_XB16", "1"))  # fp16 x tile buffering depth
TWO_Q = os.environ.get("KERNEL_2Q", "1") == "1"  # alternate SP/Act HWDGE rings
N0_DEFAULT = 65536

_bass_cache: dict = {}


def _split(n: int) -> dict:
    """Per-core row partition: nloc rows/core (R16-aligned): gp thin fp8
    prelude groups, g8 full fp8 groups, g16 full fp16 groups, kp-partition
    partial fp16 group.  Returns counts + the DVE group assignment."""
    nloc = -(-n // NC)
    nloc = -(-nloc // R16) * R16
    n8 = int(P8 * nloc / G8_ROWS + 1e-9) * G8_ROWS
    gp = min(N_PRE, n8 // GP_ROWS)
    g8, pre_rows = divmod(n8 - gp * GP_ROWS, G8_ROWS)
    # fold any sub-group fp8 residue back into the fp16 tail
    n8 -= pre_rows
    rest = nloc - n8
    g16, rem = divmod(rest, G16_ROWS)
    kp = rem // R16
    # DVE groups: early in the stream (arrivals outpace the ~18us/group
    # Vector-engine rate) with stride 3 so the PE isn't starved between
    # them; the PE drains the late groups (3.5us tail vs 18us if the
    # Vector engine owned the last-arriving group).
    k = min(N_DVE, g8)
    dve = {1 + i * 3 for i in range(k) if 1 + i * 3 < g8}
    return dict(nloc=nloc, gp=gp, g8=g8, n8=n8, g16=g16, kp=kp, dve=tuple(sorted(dve)))


def _build_bass(sp: dict) -> bass.Bass:
    gp, g8, g16, kp = sp["gp"], sp["g8"], sp["g16"], sp["kp"]
    dve = set(sp["dve"])
    kd = len(dve)
    n8 = sp["n8"]
    n16 = g16 * G16_ROWS + kp * R16
    # w slot-column layout: [prelude][fp8 main][fp16 main][fp16 remainder]
    slp = gp * R8
    sl8 = g8 * R8
    sl16 = g16 * R16 + (R16 if kp else 0)
    nmm8 = R8 // M
    nmm16 = R16 // M
    total_mm = (gp + g8 - kd) * nmm8 + g16 * nmm16 + (nmm16 if kp else 0)
    assert total_mm > 0

    nc = bacc.Bacc("TRN2", target_bir_lowering=False)
    x8_d = nc.dram_tensor("x8", [max(n8, 1) * F], FP8_BIR, kind="ExternalInput")
    x16_d = nc.dram_tensor(
        "x16", [max(n16, 1) * F], mybir.dt.float16, kind="ExternalInput"
    )
    w_d = nc.dram_tensor(
        "w", [128, slp + sl8 + sl16], mybir.dt.float16, kind="ExternalInput"
    )
    out_d = nc.dram_tensor("out", [M, M * F], mybir.dt.float32, kind="ExternalOutput")
    acc_d = nc.dram_tensor(
        "acc", [128, max(kd, 1) * F], mybir.dt.float32, kind="ExternalOutput"
    )

    mm_idx = [0]

    with TileContext(nc) as tc:
        with (
            tc.tile_pool(name="wpool", bufs=1) as wpool,
            tc.tile_pool(name="dumpool", bufs=1) as dumpool,
            tc.tile_pool(name="x8pool", bufs=XB8) as x8pool,
            tc.tile_pool(name="xdpool", bufs=XBD or max(kd, 1)) as xdpool,
            tc.tile_pool(name="x16pool", bufs=max(XB16, g16)) as x16pool,
            tc.tile_pool(name="tmppool", bufs=1) as tmppool,
            tc.tile_pool(name="accpool", bufs=1) as accpool,
            tc.tile_pool(name="ppool", bufs=1, space="PSUM") as ppool,
            tc.tile_pool(name="dppool", bufs=1, space="PSUM") as dppool,
            tc.tile_pool(name="opool", bufs=1) as opool,
        ):
            psum = ppool.tile([M, M * F], mybir.dt.float32)

            # PE warm-up: dummy matmuls on a memset tile into a scratch
            # PSUM bank, filling the idle window before the first x tile
            # lands so the HAM clock-gate releases sooner.
            if N_WARM:
                dum = dumpool.tile([1, 512], mybir.dt.float16)
                nc.gpsimd.memset(dum, 0.0)
                dpsum = dppool.tile([1, 512], mybir.dt.float32)
                for _ in range(N_WARM):
                    nc.tensor.matmul(dpsum, dum[:, :1], dum, start=True, stop=True)

            # split w: prelude slots first (tiny DMA -> early first matmul)
            w0 = w1 = None
            if slp:
                w0 = wpool.tile([128, slp], mybir.dt.float16, tag="w0")
                (nc.scalar if TWO_Q else nc.sync).dma_start(out=w0, in_=w_d[:, :slp])
            if sl8 + sl16:
                w1 = wpool.tile([128, sl8 + sl16], mybir.dt.float16, tag="w1")
                (nc.scalar if TWO_Q else nc.sync).dma_start(out=w1, in_=w_d[:, slp:])

            def mm(lhsT, rhs):
                i = mm_idx[0]
                nc.tensor.matmul(
                    psum, lhsT, rhs, start=(i == 0), stop=(i == total_mm - 1)
                )
                mm_idx[0] = i + 1

            if gp:
                xvp = x8_d[: gp * GP_ROWS * F].rearrange(
                    "(g k s) -> g k s", k=PREP, s=R8 * F
                )
                for g in range(gp):
                    eng = nc.scalar if (TWO_Q and g % 2) else nc.sync
                    xt = x8pool.tile([128, R8 * F], FP8_BIR, tag="x8")
                    eng.dma_start(out=xt[:PREP, :], in_=xvp[g])
                    for j in range(nmm8):
                        mm(
                            w0[:PREP, g * R8 + j * M : g * R8 + (j + 1) * M],
                            xt[:PREP, j * M * F : (j + 1) * M * F],
                        )
            # hoist the fp16-section DMAs ahead of the fp8 stream: their
            # data parks in dedicated SBUF tiles while bandwidth is
            # plentiful, so the PE's last inputs never arrive late (the
            # fp16 matmuls still run last, emitted after the fp8 loop)
            x16_tiles = []
            xr = None
            if g16:
                xv16 = x16_d[: g16 * G16_ROWS * F].rearrange(
                    "(g k s) -> g k s", k=128, s=R16 * F
                )
                for g in range(g16):
                    eng = nc.sync if g % 2 else (nc.scalar if TWO_Q else nc.sync)
                    xt = x16pool.tile([128, R16 * F], mybir.dt.float16, tag="x16")
                    eng.dma_start(out=xt, in_=xv16[g])
                    x16_tiles.append(xt)
            if kp:
                xr = x16pool.tile([128, R16 * F], mybir.dt.float16, tag="xr")
                nc.sync.dma_start(
                    out=xr[:kp, :],
                    in_=x16_d[g16 * G16_ROWS * F :].rearrange(
                        "(k s) -> k s", s=R16 * F
                    ),
                )
            if g8:
                xv8 = x8_d[gp * GP_ROWS * F :].rearrange(
                    "(g k s) -> g k s", k=128, s=R8 * F
                )
                di = 0
                acc_sb = None
                if kd:
                    acc_sb = accpool.tile(
                        [128, kd * F], mybir.dt.float32, tag="acc_sb", name="acc_sb"
                    )
                for g in range(g8):
                    eng = nc.scalar if (TWO_Q and (gp + g) % 2) else nc.sync
                    if g in dve:
                        xt = xdpool.tile([128, R8 * F], FP8_BIR, tag="xd")
                        eng.dma_start(out=xt, in_=xv8[g])
                        wg = w1[:, g * R8 : (g + 1) * R8]
                        tmp = tmppool.tile([128, R8 * F], mybir.dt.float16)
                        # tmp[k, t*F+f] = x[k, t*F+f] * w[k, t]
                        nc.vector.scalar_tensor_tensor(
                            out=tmp.rearrange("k (t f) -> k t f", f=F),
                            in0=xt.rearrange("k (t f) -> k t f", f=F),
                            scalar=1.0,
                            in1=wg.unsqueeze(2).broadcast_to([128, R8, F]),
                            op0=mybir.AluOpType.mult,
                            op1=mybir.AluOpType.mult,
                        )
                        # in-place pairwise tree-sum over slots: every stage
                        # is contiguous stride-1 fp16 (DVE 2x-eligible),
                        # unlike a single strided tensor_reduce (measured
                        # 13.9us vs ~5us for the tree)
                        half = (R8 // 2) * F
                        while half >= F:
                            nc.vector.scalar_tensor_tensor(
                                out=tmp[:, :half],
                                in0=tmp[:, :half],
                                scalar=1.0,
                                in1=tmp[:, half : 2 * half],
                                op0=mybir.AluOpType.mult,
                                op1=mybir.AluOpType.add,
                            )
                            half //= 2
                        nc.vector.tensor_copy(
                            acc_sb[:, di * F : (di + 1) * F], tmp[:, :F]
                        )
                        di += 1
                    else:
                        xt = x8pool.tile([128, R8 * F], FP8_BIR, tag="x8")
                        eng.dma_start(out=xt, in_=xv8[g])
                        for j in range(nmm8):
                            mm(
                                w1[:, g * R8 + j * M : g * R8 + (j + 1) * M],
                                xt[:, j * M * F : (j + 1) * M * F],
                            )
                if kd:
                    nc.sync.dma_start(out=acc_d[:, :], in_=acc_sb)
            for g, xt in enumerate(x16_tiles):
                for j in range(nmm16):
                    mm(
                        w1[
                            :,
                            sl8 + g * R16 + j * M : sl8 + g * R16 + (j + 1) * M,
                        ],
                        xt[:, j * M * F : (j + 1) * M * F],
                    )
            if kp:
                off = sl8 + g16 * R16
                for j in range(nmm16):
                    mm(
                        w1[:kp, off + j * M : off + (j + 1) * M],
                        xr[:kp, j * M * F : (j + 1) * M * F],
                    )
            out_sb = opool.tile([M, M * F], mybir.dt.float32)
            nc.vector.tensor_copy(out_sb, psum)
            nc.sync.dma_start(out=out_d[:, :], in_=out_sb)
    nc.compile()
    return nc


def _get_bass(sp: dict) -> bass.Bass:
    key = (
        sp["gp"], sp["g8"], sp["g16"], sp["kp"], sp["dve"],
        R8, XB8, XBD, XB16, TWO_Q, N_WARM,
    )  # fmt: skip
    if key not in _bass_cache:
        _bass_cache[key] = _build_bass(sp)
    return _bass_cache[key]


def _slot_major(wc: np.ndarray, g: int, p: int, r: int) -> np.ndarray:
    """[g*p*r] row-weights -> [128, g*r] slot-major (partition, g*r+t),
    zero-padded from p to 128 partitions."""
    if not g:
        return np.zeros((128, 0), wc.dtype)
    m = np.ascontiguousarray(wc.reshape(g, p, r).transpose(1, 0, 2)).reshape(p, g * r)
    return np.pad(m, ((0, 128 - p), (0, 0))) if p < 128 else m


def _run(x: np.ndarray, w: np.ndarray, trace: bool = False, tmpdir=None):
    """Shard x [n, 64] fp32 + per-row weights w [n] (fp64) over 8 cores,
    return (weighted row-sum [64] as float64, BassKernelResults)."""
    n = x.shape[0]
    sp = _split(n)
    nloc, gp, g8, n8, g16, kp = (
        sp["nloc"], sp["gp"], sp["g8"], sp["n8"], sp["g16"], sp["kp"],
    )  # fmt: skip
    kd = len(sp["dve"])
    n16 = nloc - n8

    in_maps = []
    for c in range(NC):
        lo = c * nloc
        hi = min(lo + nloc, n)
        wc = np.zeros(nloc, np.float16)
        wc[: max(hi - lo, 0)] = w[lo:hi]
        xc8 = np.zeros((n8, F), FP8_NP)
        xc16 = np.zeros((n16, F), np.float16)
        if hi > lo:
            m8 = min(n8, hi - lo)
            xc8[:m8] = x[lo : lo + m8].astype(FP8_NP)
            if hi > lo + n8:
                xc16[: hi - lo - n8] = x[lo + n8 : hi].astype(np.float16)
        npre = gp * GP_ROWS
        wmap = np.concatenate(
            [
                _slot_major(wc[:npre], gp, PREP, R8),
                _slot_major(wc[npre:n8], g8, 128, R8),
                _slot_major(wc[n8 : n8 + g16 * G16_ROWS], g16, 128, R16),
            ]
            + (
                [
                    np.pad(
                        wc[n8 + g16 * G16_ROWS :].reshape(kp, R16),
                        ((0, 128 - kp), (0, 0)),
                    )
                ]
                if kp
                else []
            ),
            axis=1,
        )
        in_maps.append(
            {
                "x8": xc8.reshape(-1) if n8 else np.zeros(F, FP8_NP),
                "x16": xc16.reshape(-1) if n16 else np.zeros(F, np.float16),
                "w": wmap,
            }
        )

    nc = _get_bass(sp)
    res = run_bass_kernel_spmd(
        nc, in_maps, core_ids=list(range(NC)), trace=trace, tmpdir=tmpdir
    )
    total = np.zeros(F, np.float64)
    for c in range(NC):
        o = np.asarray(res.results[c]["out"], np.float64)  # [M, M*F]
        for t in range(M):
            total += o[t, t * F : (t + 1) * F]
        if kd:
            a = np.asarray(res.results[c]["acc"], np.float64)  # [128, kd*F]
            total += a.reshape(128, kd, F).sum(axis=(0, 1))
    return total, res


def kernel(x_atom_fea, segment_ids, num_segments=None, **_ignored):
    x = np.asarray(x_atom_fea, dtype=np.float32)
    seg = np.asarray(segment_ids).astype(np.int64, copy=False)
    n0 = int(num_segments) if num_segments is not None else N0_DEFAULT
    counts = np.bincount(seg, minlength=n0)
    # w = 1/count stays in fp16's *normal* range; the 1/N0 factor would
    # push it subnormal and wreck precision, so divide by N0 on the host
    # after the device reduction instead.
    w = 1.0 / np.maximum(counts, 1).astype(np.float64)
    total, _ = _run(x, w[seg])
    return (total / float(n0)).astype(np.float32).reshape(1, F)


# revision 32
# speedup vs baseline: 1.1100x; 1.1100x over previous
"""Segment-mean-of-means kernel for Trainium2 (8 NeuronCores, SPMD).

Problem: out = mean_s( segment_sum(x)[s] / max(count_s, 1) ) over 65536
segments of a [4M, 64] fp32 tensor with *sorted* segment ids.

Reformulation: out[f] = (1/N0) * sum_i w_i * x_i[f] with w_i =
1/count_{seg(i)} -- a pure streaming weighted row-reduction.  The device
is HBM-bound, so x is streamed as fp8 E3M4 (host-side cast; 4 mantissa
bits -> 1.7e-2 max-rel on the target data vs the 2e-2 budget) with an
optional fp16 tail (P8 dial).  Weights stay fp16 (mixed-dtype matmul).

Device structure per core:
  - prelude: a few thin groups (32 partitions x R8 slots) so the first
    matmul only waits on a 256KB DMA instead of 1MB+w.
  - main fp8 groups (128 x R8): most go through PE matmuls accumulating
    into one PSUM bank; a few are offloaded to the otherwise-idle Vector
    engine (broadcast-multiply into a tmp tile, then a strided reduce
    into a per-group fp32 slab) to shorten the PE stream.
  - fp16 groups + a partial remainder group close out the rows.
Host sums the PSUM diagonal blocks + the DVE slab partitions, then
divides by N0.

Group layout: row j of a group lives at (partition k = j//R, slot t =
j%R); a partition's slice of a group is R*64 contiguous bytes in DRAM
(8KB at R8=128 fp8), one DMA descriptor.  Each PE group is reduced by
R/8 matmuls  lhsT = w[:, off+g*R+8j : +8] (fp16), rhs = x_sb[:,
8j*64:(8j+8)*64] -> psum[8, 512], whose diagonal 64-blocks accumulate
the weighted sums (off-diagonal blocks are garbage ignored on the host).
"""

import os

import ml_dtypes
import numpy as np

import concourse.bass as bass
import concourse.mybir as mybir
from concourse import bacc
from concourse.bass_utils import run_bass_kernel_spmd
from concourse.tile import TileContext


def _harden_trace_path():
    """If a caller enables tracing (e.g. BASS_TRACE=1), run_bass_kernel_spmd
    imports antenv.axon_hooks, which this image lacks -- that would crash the
    run.  Provide the hook via trn_boot's ctypes shim (or a None hook, which
    bass_utils degrades on gracefully), and make the artifact upload failure
    non-fatal (zero-egress sandbox)."""
    import sys
    import types

    try:
        import antenv.axon_hooks  # noqa: F401  # already provided: nothing to do
        return
    except ImportError:
        pass
    hook = None
    try:
        import trn_agent_boot.trn_boot as tb

        hook = tb._ntff_profile_via_ctypes("/opt/axon/libaxon_pjrt.so")
    except Exception:
        pass
    mod = types.ModuleType("antenv.axon_hooks")
    mod.get_axon_ntff_profile_hook = lambda: hook
    sys.modules["antenv.axon_hooks"] = mod

    import concourse.bass_utils as bu

    _orig_upload = bu.upload_artifacts

    def _safe_upload(tmpdir):
        try:
            return _orig_upload(tmpdir)
        except Exception:
            return tmpdir

    bu.upload_artifacts = _safe_upload


_harden_trace_path()

F = 64  # features
NC = 8  # cores
M = 8  # matmul M dim (psum partitions); 8*F = 512 = one PSUM bank
FP8_NP = ml_dtypes.float8_e3m4
FP8_BIR = mybir.dt.float8e3

P8 = float(os.environ.get("KERNEL_P8", "1.0"))  # fraction of rows in e3m4
R8 = int(os.environ.get("KERNEL_R8", "128"))  # fp8 rows/partition/group (8KB runs)
R16 = 64  # fp16 rows/partition/group (8KB runs)
PREP = 64  # prelude partitions (fewer descriptors -> faster first DMA)
G8_ROWS = 128 * R8  # rows per fp8 main group
GP_ROWS = PREP * R8  # rows per prelude group
G16_ROWS = 128 * R16  # rows per fp16 group
N_PRE = int(os.environ.get("KERNEL_PRE", "2"))  # thin prelude groups
N_DVE = int(os.environ.get("KERNEL_DVE", "0"))  # main fp8 groups on Vector engine
N_WARM = int(os.environ.get("KERNEL_WARM", "5"))  # PE warm-up dummy matmuls
XB8 = int(os.environ.get("KERNEL_XB8", "9"))  # fp8 x tile buffering depth
# DVE x tiles: one buffer per offloaded group, so a DVE tile's dma_start
# never waits on the slow Vector consumer to free a buffer (such a wait
# head-of-line-blocks the whole in-order DMA issue queue).
XBD = int(os.environ.get("KERNEL_XBD", "0")) or None
XB16 = int(os.environ.get("KERNEL_XB16", "1"))  # fp16 x tile buffering depth
TWO_Q = os.environ.get("KERNEL_2Q", "1") == "1"  # alternate SP/Act HWDGE rings
N0_DEFAULT = 65536

_bass_cache: dict = {}


def _split(n: int) -> dict:
    """Per-core row partition: nloc rows/core (R16-aligned): gp thin fp8
    prelude groups, g8 full fp8 groups, g16 full fp16 groups, kp-partition
    partial fp16 group.  Returns counts + the DVE group assignment."""
    nloc = -(-n // NC)
    nloc = -(-nloc // R16) * R16
    n8 = int(P8 * nloc / G8_ROWS + 1e-9) * G8_ROWS
    gp = min(N_PRE, n8 // GP_ROWS)
    g8, pre_rows = divmod(n8 - gp * GP_ROWS, G8_ROWS)
    # fold any sub-group fp8 residue back into the fp16 tail
    n8 -= pre_rows
    rest = nloc - n8
    g16, rem = divmod(rest, G16_ROWS)
    kp = rem // R16
    # DVE groups: early in the stream (arrivals outpace the ~18us/group
    # Vector-engine rate) with stride 3 so the PE isn't starved between
    # them; the PE drains the late groups (3.5us tail vs 18us if the
    # Vector engine owned the last-arriving group).
    k = min(N_DVE, g8)
    dve = {1 + i * 3 for i in range(k) if 1 + i * 3 < g8}
    return dict(nloc=nloc, gp=gp, g8=g8, n8=n8, g16=g16, kp=kp, dve=tuple(sorted(dve)))


def _build_bass(sp: dict) -> bass.Bass:
    gp, g8, g16, kp = sp["gp"], sp["g8"], sp["g16"], sp["kp"]
    dve = set(sp["dve"])
    kd = len(dve)
    n8 = sp["n8"]
    n16 = g16 * G16_ROWS + kp * R16
    # w slot-column layout: [prelude][fp8 main][fp16 main][fp16 remainder]
    slp = gp * R8
    sl8 = g8 * R8
    sl16 = g16 * R16 + (R16 if kp else 0)
    nmm8 = R8 // M
    nmm16 = R16 // M
    # PE units in emission order; units alternate between the two PE
    # column-tile regions (tile_position (0,0) / (0,32)), which stream
    # concurrently through different 32-column strips of the array
    unit_nmm = (
        [nmm8] * gp + [nmm8] * (g8 - kd) + [nmm16] * g16 + ([nmm16] if kp else [])
    )
    region_total = [
        sum(n for u, n in enumerate(unit_nmm) if u % 2 == r) for r in (0, 1)
    ]
    assert sum(region_total) > 0

    nc = bacc.Bacc("TRN2", target_bir_lowering=False)
    x8_d = nc.dram_tensor("x8", [max(n8, 1) * F], FP8_BIR, kind="ExternalInput")
    x16_d = nc.dram_tensor(
        "x16", [max(n16, 1) * F], mybir.dt.float16, kind="ExternalInput"
    )
    w_d = nc.dram_tensor(
        "w", [128, slp + sl8 + sl16], mybir.dt.float16, kind="ExternalInput"
    )
    out_d = nc.dram_tensor(
        "out", [32 + M, M * F], mybir.dt.float32, kind="ExternalOutput"
    )
    acc_d = nc.dram_tensor(
        "acc", [128, max(kd, 1) * F], mybir.dt.float32, kind="ExternalOutput"
    )

    mm_idx = [0]

    with TileContext(nc) as tc:
        with (
            tc.tile_pool(name="wpool", bufs=1) as wpool,
            tc.tile_pool(name="dumpool", bufs=1) as dumpool,
            tc.tile_pool(name="x8pool", bufs=XB8) as x8pool,
            tc.tile_pool(name="xdpool", bufs=XBD or max(kd, 1)) as xdpool,
            tc.tile_pool(name="x16pool", bufs=max(XB16, g16)) as x16pool,
            tc.tile_pool(name="tmppool", bufs=1) as tmppool,
            tc.tile_pool(name="accpool", bufs=1) as accpool,
            tc.tile_pool(name="ppool", bufs=1, space="PSUM") as ppool,
            tc.tile_pool(name="dppool", bufs=1, space="PSUM") as dppool,
            tc.tile_pool(name="opool", bufs=1) as opool,
        ):
            psum = ppool.tile([32 + M, M * F], mybir.dt.float32)

            # PE warm-up: dummy matmuls on a memset tile into a scratch
            # PSUM bank, filling the idle window before the first x tile
            # lands so the HAM clock-gate releases sooner.
            if N_WARM:
                dum = dumpool.tile([1, 512], mybir.dt.float16)
                nc.gpsimd.memset(dum, 0.0)
                dpsum = dppool.tile([1, 512], mybir.dt.float32)
                for _ in range(N_WARM):
                    nc.tensor.matmul(dpsum, dum[:, :1], dum, start=True, stop=True)

            # split w: prelude slots first (tiny DMA -> early first matmul)
            w0 = w1 = None
            if slp:
                w0 = wpool.tile([128, slp], mybir.dt.float16, tag="w0")
                (nc.scalar if TWO_Q else nc.sync).dma_start(out=w0, in_=w_d[:, :slp])
            if sl8 + sl16:
                w1 = wpool.tile([128, sl8 + sl16], mybir.dt.float16, tag="w1")
                (nc.scalar if TWO_Q else nc.sync).dma_start(out=w1, in_=w_d[:, slp:])

            # paired-unit emission: units go alternately to PE column-tile
            # regions 0 / 1 (tile_position (0,0) / (0,32), psum partitions
            # 0:M / 32:32+M); a pair's matmuls are interleaved j-wise so
            # the two streams overlap in different 32-column strips of the
            # PE array (~2x effective moving-operand rate)
            unit_idx = [0]
            region_cnt = [0, 0]
            pending = []

            def emit_mm(r, w_ap, x_ap):
                i = region_cnt[r]
                nc.tensor.matmul(
                    psum[32 * r : 32 * r + M, :],
                    w_ap,
                    x_ap,
                    start=(i == 0),
                    stop=(i == region_total[r] - 1),
                    tile_position=(0, 32 * r),
                    skip_group_check=True,
                )
                region_cnt[r] += 1

            def mm_unit(w_ap, x_ap, nmm):
                pending.append((w_ap, x_ap, nmm))
                if len(pending) < 2:
                    return
                (w_a, x_a, n_a), (w_b, x_b, n_b) = pending
                pending.clear()
                ra = unit_idx[0] % 2
                unit_idx[0] += 2
                for j in range(max(n_a, n_b)):
                    if j < n_a:
                        emit_mm(ra, w_a(j), x_a(j))
                    if j < n_b:
                        emit_mm(1 - ra, w_b(j), x_b(j))

            def mm_flush():
                if pending:
                    w_a, x_a, n_a = pending.pop()
                    r = unit_idx[0] % 2
                    unit_idx[0] += 1
                    for j in range(n_a):
                        emit_mm(r, w_a(j), x_a(j))

            if gp:
                xvp = x8_d[: gp * GP_ROWS * F].rearrange(
                    "(g k s) -> g k s", k=PREP, s=R8 * F
                )
                for g in range(gp):
                    eng = nc.scalar if (TWO_Q and g % 2) else nc.sync
                    xt = x8pool.tile([128, R8 * F], FP8_BIR, tag="x8")
                    eng.dma_start(out=xt[:PREP, :], in_=xvp[g])
                    mm_unit(
                        lambda j, g=g: w0[:PREP, g * R8 + j * M : g * R8 + (j + 1) * M],
                        lambda j, xt=xt: xt[:PREP, j * M * F : (j + 1) * M * F],
                        nmm8,
                    )
            # hoist the fp16-section DMAs ahead of the fp8 stream: their
            # data parks in dedicated SBUF tiles while bandwidth is
            # plentiful, so the PE's last inputs never arrive late (the
            # fp16 matmuls still run last, emitted after the fp8 loop)
            x16_tiles = []
            xr = None
            if g16:
                xv16 = x16_d[: g16 * G16_ROWS * F].rearrange(
                    "(g k s) -> g k s", k=128, s=R16 * F
                )
                for g in range(g16):
                    eng = nc.sync if g % 2 else (nc.scalar if TWO_Q else nc.sync)
                    xt = x16pool.tile([128, R16 * F], mybir.dt.float16, tag="x16")
                    eng.dma_start(out=xt, in_=xv16[g])
                    x16_tiles.append(xt)
            if kp:
                xr = x16pool.tile([128, R16 * F], mybir.dt.float16, tag="xr")
                nc.sync.dma_start(
                    out=xr[:kp, :],
                    in_=x16_d[g16 * G16_ROWS * F :].rearrange(
                        "(k s) -> k s", s=R16 * F
                    ),
                )
            if g8:
                xv8 = x8_d[gp * GP_ROWS * F :].rearrange(
                    "(g k s) -> g k s", k=128, s=R8 * F
                )
                di = 0
                acc_sb = None
                if kd:
                    acc_sb = accpool.tile(
                        [128, kd * F], mybir.dt.float32, tag="acc_sb", name="acc_sb"
                    )
                for g in range(g8):
                    eng = nc.scalar if (TWO_Q and (gp + g) % 2) else nc.sync
                    if g in dve:
                        xt = xdpool.tile([128, R8 * F], FP8_BIR, tag="xd")
                        eng.dma_start(out=xt, in_=xv8[g])
                        wg = w1[:, g * R8 : (g + 1) * R8]
                        tmp = tmppool.tile([128, R8 * F], mybir.dt.float16)
                        # tmp[k, t*F+f] = x[k, t*F+f] * w[k, t]
                        nc.vector.scalar_tensor_tensor(
                            out=tmp.rearrange("k (t f) -> k t f", f=F),
                            in0=xt.rearrange("k (t f) -> k t f", f=F),
                            scalar=1.0,
                            in1=wg.unsqueeze(2).broadcast_to([128, R8, F]),
                            op0=mybir.AluOpType.mult,
                            op1=mybir.AluOpType.mult,
                        )
                        # in-place pairwise tree-sum over slots: every stage
                        # is contiguous stride-1 fp16 (DVE 2x-eligible),
                        # unlike a single strided tensor_reduce (measured
                        # 13.9us vs ~5us for the tree)
                        half = (R8 // 2) * F
                        while half >= F:
                            nc.vector.scalar_tensor_tensor(
                                out=tmp[:, :half],
                                in0=tmp[:, :half],
                                scalar=1.0,
                                in1=tmp[:, half : 2 * half],
                                op0=mybir.AluOpType.mult,
                                op1=mybir.AluOpType.add,
                            )
                            half //= 2
                        nc.vector.tensor_copy(
                            acc_sb[:, di * F : (di + 1) * F], tmp[:, :F]
                        )
                        di += 1
                    else:
                        xt = x8pool.tile([128, R8 * F], FP8_BIR, tag="x8")
                        eng.dma_start(out=xt, in_=xv8[g])
                        mm_unit(
                            lambda j, g=g: w1[
                                :, g * R8 + j * M : g * R8 + (j + 1) * M
                            ],
                            lambda j, xt=xt: xt[:, j * M * F : (j + 1) * M * F],
                            nmm8,
                        )
                if kd:
                    nc.sync.dma_start(out=acc_d[:, :], in_=acc_sb)
            for g, xt in enumerate(x16_tiles):
                mm_unit(
                    lambda j, g=g: w1[
                        :, sl8 + g * R16 + j * M : sl8 + g * R16 + (j + 1) * M
                    ],
                    lambda j, xt=xt: xt[:, j * M * F : (j + 1) * M * F],
                    nmm16,
                )
            if kp:
                off = sl8 + g16 * R16
                mm_unit(
                    lambda j: w1[:kp, off + j * M : off + (j + 1) * M],
                    lambda j: xr[:kp, j * M * F : (j + 1) * M * F],
                    nmm16,
                )
            mm_flush()
            out_sb = opool.tile([32 + M, M * F], mybir.dt.float32)
            nc.vector.tensor_copy(out_sb, psum)
            nc.sync.dma_start(out=out_d[:, :], in_=out_sb)
    nc.compile()
    return nc


def _get_bass(sp: dict) -> bass.Bass:
    key = (
        sp["gp"], sp["g8"], sp["g16"], sp["kp"], sp["dve"],
        R8, XB8, XBD, XB16, TWO_Q, N_WARM,
    )  # fmt: skip
    if key not in _bass_cache:
        _bass_cache[key] = _build_bass(sp)
    return _bass_cache[key]


def _slot_major(wc: np.ndarray, g: int, p: int, r: int) -> np.ndarray:
    """[g*p*r] row-weights -> [128, g*r] slot-major (partition, g*r+t),
    zero-padded from p to 128 partitions."""
    if not g:
        return np.zeros((128, 0), wc.dtype)
    m = np.ascontiguousarray(wc.reshape(g, p, r).transpose(1, 0, 2)).reshape(p, g * r)
    return np.pad(m, ((0, 128 - p), (0, 0))) if p < 128 else m


def _run(x: np.ndarray, w: np.ndarray, trace: bool = False, tmpdir=None):
    """Shard x [n, 64] fp32 + per-row weights w [n] (fp64) over 8 cores,
    return (weighted row-sum [64] as float64, BassKernelResults)."""
    n = x.shape[0]
    sp = _split(n)
    nloc, gp, g8, n8, g16, kp = (
        sp["nloc"], sp["gp"], sp["g8"], sp["n8"], sp["g16"], sp["kp"],
    )  # fmt: skip
    kd = len(sp["dve"])
    n16 = nloc - n8

    in_maps = []
    for c in range(NC):
        lo = c * nloc
        hi = min(lo + nloc, n)
        wc = np.zeros(nloc, np.float16)
        wc[: max(hi - lo, 0)] = w[lo:hi]
        xc8 = np.zeros((n8, F), FP8_NP)
        xc16 = np.zeros((n16, F), np.float16)
        if hi > lo:
            m8 = min(n8, hi - lo)
            xc8[:m8] = x[lo : lo + m8].astype(FP8_NP)
            if hi > lo + n8:
                xc16[: hi - lo - n8] = x[lo + n8 : hi].astype(np.float16)
        npre = gp * GP_ROWS
        wmap = np.concatenate(
            [
                _slot_major(wc[:npre], gp, PREP, R8),
                _slot_major(wc[npre:n8], g8, 128, R8),
                _slot_major(wc[n8 : n8 + g16 * G16_ROWS], g16, 128, R16),
            ]
            + (
                [
                    np.pad(
                        wc[n8 + g16 * G16_ROWS :].reshape(kp, R16),
                        ((0, 128 - kp), (0, 0)),
                    )
                ]
                if kp
                else []
            ),
            axis=1,
        )
        in_maps.append(
            {
                "x8": xc8.reshape(-1) if n8 else np.zeros(F, FP8_NP),
                "x16": xc16.reshape(-1) if n16 else np.zeros(F, np.float16),
                "w": wmap,
            }
        )

    nc = _get_bass(sp)
    res = run_bass_kernel_spmd(
        nc, in_maps, core_ids=list(range(NC)), trace=trace, tmpdir=tmpdir
    )
    n_units = gp + (g8 - kd) + g16 + (1 if kp else 0)
    total = np.zeros(F, np.float64)
    for c in range(NC):
        o = np.asarray(res.results[c]["out"], np.float64)  # [32+M, M*F]
        for t in range(M):
            total += o[t, t * F : (t + 1) * F]
            if n_units >= 2:  # PE column-tile region 1 in psum rows 32:32+M
                total += o[32 + t, t * F : (t + 1) * F]
        if kd:
            a = np.asarray(res.results[c]["acc"], np.float64)  # [128, kd*F]
            total += a.reshape(128, kd, F).sum(axis=(0, 1))
    return total, res


def kernel(x_atom_fea, segment_ids, num_segments=None, **_ignored):
    x = np.asarray(x_atom_fea, dtype=np.float32)
    seg = np.asarray(segment_ids).astype(np.int64, copy=False)
    n0 = int(num_segments) if num_segments is not None else N0_DEFAULT
    counts = np.bincount(seg, minlength=n0)
    # w = 1/count stays in fp16's *normal* range; the 1/N0 factor would
    # push it subnormal and wreck precision, so divide by N0 on the host
    # after the device reduction instead.
    w = 1.0 / np.maximum(counts, 1).astype(np.float64)
    total, _ = _run(x, w[seg])
    return (total / float(n0)).astype(np.float32).reshape(1, F)


# revision 35
# speedup vs baseline: 1.1277x; 1.0159x over previous
"""Segment-mean-of-means kernel for Trainium2 (8 NeuronCores, SPMD).

Problem: out = mean_s( segment_sum(x)[s] / max(count_s, 1) ) over 65536
segments of a [4M, 64] fp32 tensor with *sorted* segment ids.

Reformulation: out[f] = (1/N0) * sum_i w_i * x_i[f] with w_i =
1/count_{seg(i)} -- a pure streaming weighted row-reduction.  The device
is HBM-bound, so x is streamed as fp8 E3M4 (host-side cast; 4 mantissa
bits -> 1.7e-2 max-rel on the target data vs the 2e-2 budget) with an
optional fp16 tail (P8 dial).  Weights stay fp16 (mixed-dtype matmul).

Device structure per core:
  - prelude: a few thin groups (32 partitions x R8 slots) so the first
    matmul only waits on a 256KB DMA instead of 1MB+w.
  - main fp8 groups (128 x R8): most go through PE matmuls accumulating
    into one PSUM bank; a few are offloaded to the otherwise-idle Vector
    engine (broadcast-multiply into a tmp tile, then a strided reduce
    into a per-group fp32 slab) to shorten the PE stream.
  - fp16 groups + a partial remainder group close out the rows.
Host sums the PSUM diagonal blocks + the DVE slab partitions, then
divides by N0.

Group layout: row j of a group lives at (partition k = j//R, slot t =
j%R); a partition's slice of a group is R*64 contiguous bytes in DRAM
(8KB at R8=128 fp8), one DMA descriptor.  Each PE group is reduced by
R/8 matmuls  lhsT = w[:, off+g*R+8j : +8] (fp16), rhs = x_sb[:,
8j*64:(8j+8)*64] -> psum[8, 512], whose diagonal 64-blocks accumulate
the weighted sums (off-diagonal blocks are garbage ignored on the host).
"""

import os

import ml_dtypes
import numpy as np

import concourse.bass as bass
import concourse.mybir as mybir
from concourse import bacc
from concourse.bass_utils import run_bass_kernel_spmd
from concourse.tile import TileContext


def _harden_trace_path():
    """If a caller enables tracing (e.g. BASS_TRACE=1), run_bass_kernel_spmd
    imports antenv.axon_hooks, which this image lacks -- that would crash the
    run.  Provide the hook via trn_boot's ctypes shim (or a None hook, which
    bass_utils degrades on gracefully), and make the artifact upload failure
    non-fatal (zero-egress sandbox)."""
    import sys
    import types

    try:
        import antenv.axon_hooks  # noqa: F401  # already provided: nothing to do
        return
    except ImportError:
        pass
    hook = None
    try:
        import trn_agent_boot.trn_boot as tb

        hook = tb._ntff_profile_via_ctypes("/opt/axon/libaxon_pjrt.so")
    except Exception:
        pass
    mod = types.ModuleType("antenv.axon_hooks")
    mod.get_axon_ntff_profile_hook = lambda: hook
    sys.modules["antenv.axon_hooks"] = mod

    import concourse.bass_utils as bu

    _orig_upload = bu.upload_artifacts

    def _safe_upload(tmpdir):
        try:
            return _orig_upload(tmpdir)
        except Exception:
            return tmpdir

    bu.upload_artifacts = _safe_upload


_harden_trace_path()

F = 64  # features
NC = 8  # cores
M = 8  # matmul M dim (psum partitions); 8*F = 512 = one PSUM bank
FP8_NP = ml_dtypes.float8_e3m4
FP8_BIR = mybir.dt.float8e3

P8 = float(os.environ.get("KERNEL_P8", "1.0"))  # fraction of rows in e3m4
R8 = int(os.environ.get("KERNEL_R8", "128"))  # fp8 rows/partition/group (8KB runs)
R16 = 64  # fp16 rows/partition/group (8KB runs)
PREP = 64  # prelude partitions (fewer descriptors -> faster first DMA)
G8_ROWS = 128 * R8  # rows per fp8 main group
GP_ROWS = PREP * R8  # rows per prelude group
G16_ROWS = 128 * R16  # rows per fp16 group
N_PRE = int(os.environ.get("KERNEL_PRE", "2"))  # thin prelude groups
N_DVE = int(os.environ.get("KERNEL_DVE", "0"))  # main fp8 groups on Vector engine
N_WARM = int(os.environ.get("KERNEL_WARM", "5"))  # PE warm-up dummy matmuls
XB8 = int(os.environ.get("KERNEL_XB8", "14"))  # fp8 x tile buffering depth
# DVE x tiles: one buffer per offloaded group, so a DVE tile's dma_start
# never waits on the slow Vector consumer to free a buffer (such a wait
# head-of-line-blocks the whole in-order DMA issue queue).
XBD = int(os.environ.get("KERNEL_XBD", "0")) or None
XB16 = int(os.environ.get("KERNEL_XB16", "1"))  # fp16 x tile buffering depth
TWO_Q = os.environ.get("KERNEL_2Q", "1") == "1"  # alternate SP/Act HWDGE rings
N0_DEFAULT = 65536

_bass_cache: dict = {}


def _split(n: int) -> dict:
    """Per-core row partition: nloc rows/core (R16-aligned): gp thin fp8
    prelude groups, g8 full fp8 groups, g16 full fp16 groups, kp-partition
    partial fp16 group.  Returns counts + the DVE group assignment."""
    nloc = -(-n // NC)
    nloc = -(-nloc // R16) * R16
    n8 = int(P8 * nloc / G8_ROWS + 1e-9) * G8_ROWS
    gp = min(N_PRE, n8 // GP_ROWS)
    g8, pre_rows = divmod(n8 - gp * GP_ROWS, G8_ROWS)
    # fold any sub-group fp8 residue back into the fp16 tail
    n8 -= pre_rows
    rest = nloc - n8
    g16, rem = divmod(rest, G16_ROWS)
    kp = rem // R16
    # DVE groups: early in the stream (arrivals outpace the ~18us/group
    # Vector-engine rate) with stride 3 so the PE isn't starved between
    # them; the PE drains the late groups (3.5us tail vs 18us if the
    # Vector engine owned the last-arriving group).
    k = min(N_DVE, g8)
    dve = {1 + i * 3 for i in range(k) if 1 + i * 3 < g8}
    return dict(nloc=nloc, gp=gp, g8=g8, n8=n8, g16=g16, kp=kp, dve=tuple(sorted(dve)))


def _build_bass(sp: dict) -> bass.Bass:
    gp, g8, g16, kp = sp["gp"], sp["g8"], sp["g16"], sp["kp"]
    dve = set(sp["dve"])
    kd = len(dve)
    n8 = sp["n8"]
    n16 = g16 * G16_ROWS + kp * R16
    # w slot-column layout: [prelude][fp8 main][fp16 main][fp16 remainder]
    slp = gp * R8
    sl8 = g8 * R8
    sl16 = g16 * R16 + (R16 if kp else 0)
    nmm8 = R8 // M
    nmm16 = R16 // M
    # PE units in emission order; units alternate between the two PE
    # column-tile regions (tile_position (0,0) / (0,32)), which stream
    # concurrently through different 32-column strips of the array
    unit_nmm = (
        [nmm8] * gp + [nmm8] * (g8 - kd) + [nmm16] * g16 + ([nmm16] if kp else [])
    )
    region_total = [
        sum(n for u, n in enumerate(unit_nmm) if u % 2 == r) for r in (0, 1)
    ]
    assert sum(region_total) > 0

    nc = bacc.Bacc("TRN2", target_bir_lowering=False)
    x8_d = nc.dram_tensor("x8", [max(n8, 1) * F], FP8_BIR, kind="ExternalInput")
    x16_d = nc.dram_tensor(
        "x16", [max(n16, 1) * F], mybir.dt.float16, kind="ExternalInput"
    )
    w_d = nc.dram_tensor(
        "w", [128, slp + sl8 + sl16], mybir.dt.float16, kind="ExternalInput"
    )
    out_d = nc.dram_tensor(
        "out", [32 + M, M * F], mybir.dt.float32, kind="ExternalOutput"
    )
    acc_d = nc.dram_tensor(
        "acc", [128, max(kd, 1) * F], mybir.dt.float32, kind="ExternalOutput"
    )

    mm_idx = [0]

    with TileContext(nc) as tc:
        with (
            tc.tile_pool(name="wpool", bufs=1) as wpool,
            tc.tile_pool(name="dumpool", bufs=1) as dumpool,
            tc.tile_pool(name="x8pool", bufs=XB8) as x8pool,
            tc.tile_pool(name="xdpool", bufs=XBD or max(kd, 1)) as xdpool,
            tc.tile_pool(name="x16pool", bufs=max(XB16, g16)) as x16pool,
            tc.tile_pool(name="tmppool", bufs=1) as tmppool,
            tc.tile_pool(name="accpool", bufs=1) as accpool,
            tc.tile_pool(name="ppool", bufs=1, space="PSUM") as ppool,
            tc.tile_pool(name="dppool", bufs=1, space="PSUM") as dppool,
            tc.tile_pool(name="opool", bufs=1) as opool,
        ):
            psum = ppool.tile([32 + M, M * F], mybir.dt.float32)

            # PE warm-up: dummy matmuls on a memset tile into a scratch
            # PSUM bank, filling the idle window before the first x tile
            # lands so the HAM clock-gate releases sooner.
            if N_WARM:
                dum = dumpool.tile([1, 512], mybir.dt.float16)
                nc.gpsimd.memset(dum, 0.0)
                dpsum = dppool.tile([1, 512], mybir.dt.float32)
                for _ in range(N_WARM):
                    nc.tensor.matmul(dpsum, dum[:, :1], dum, start=True, stop=True)

            # split w: prelude slots first (tiny DMA -> early first matmul)
            w0 = w1 = None
            if slp:
                w0 = wpool.tile([128, slp], mybir.dt.float16, tag="w0")
                (nc.scalar if TWO_Q else nc.sync).dma_start(out=w0, in_=w_d[:, :slp])
            if sl8 + sl16:
                w1 = wpool.tile([128, sl8 + sl16], mybir.dt.float16, tag="w1")
                (nc.scalar if TWO_Q else nc.sync).dma_start(out=w1, in_=w_d[:, slp:])

            # paired-unit emission: units go alternately to PE column-tile
            # regions 0 / 1 (tile_position (0,0) / (0,32), psum partitions
            # 0:M / 32:32+M); a pair's matmuls are interleaved j-wise so
            # the two streams overlap in different 32-column strips of the
            # PE array (~2x effective moving-operand rate)
            unit_idx = [0]
            region_cnt = [0, 0]
            pending = []

            def emit_mm(r, w_ap, x_ap):
                i = region_cnt[r]
                nc.tensor.matmul(
                    psum[32 * r : 32 * r + M, :],
                    w_ap,
                    x_ap,
                    start=(i == 0),
                    stop=(i == region_total[r] - 1),
                    tile_position=(0, 32 * r),
                    skip_group_check=True,
                )
                region_cnt[r] += 1

            def mm_unit(w_ap, x_ap, nmm):
                pending.append((w_ap, x_ap, nmm))
                if len(pending) < 2:
                    return
                (w_a, x_a, n_a), (w_b, x_b, n_b) = pending
                pending.clear()
                ra = unit_idx[0] % 2
                unit_idx[0] += 2
                for j in range(max(n_a, n_b)):
                    if j < n_a:
                        emit_mm(ra, w_a(j), x_a(j))
                    if j < n_b:
                        emit_mm(1 - ra, w_b(j), x_b(j))

            def mm_flush():
                if pending:
                    w_a, x_a, n_a = pending.pop()
                    r = unit_idx[0] % 2
                    unit_idx[0] += 1
                    for j in range(n_a):
                        emit_mm(r, w_a(j), x_a(j))

            if gp:
                xvp = x8_d[: gp * GP_ROWS * F].rearrange(
                    "(g k s) -> g k s", k=PREP, s=R8 * F
                )
                for g in range(gp):
                    eng = nc.scalar if (TWO_Q and g % 2) else nc.sync
                    xt = x8pool.tile([128, R8 * F], FP8_BIR, tag="x8")
                    eng.dma_start(out=xt[:PREP, :], in_=xvp[g])
                    mm_unit(
                        lambda j, g=g: w0[:PREP, g * R8 + j * M : g * R8 + (j + 1) * M],
                        lambda j, xt=xt: xt[:PREP, j * M * F : (j + 1) * M * F],
                        nmm8,
                    )
            # hoist the fp16-section DMAs ahead of the fp8 stream: their
            # data parks in dedicated SBUF tiles while bandwidth is
            # plentiful, so the PE's last inputs never arrive late (the
            # fp16 matmuls still run last, emitted after the fp8 loop)
            x16_tiles = []
            xr = None
            if g16:
                xv16 = x16_d[: g16 * G16_ROWS * F].rearrange(
                    "(g k s) -> g k s", k=128, s=R16 * F
                )
                for g in range(g16):
                    eng = nc.sync if g % 2 else (nc.scalar if TWO_Q else nc.sync)
                    xt = x16pool.tile([128, R16 * F], mybir.dt.float16, tag="x16")
                    eng.dma_start(out=xt, in_=xv16[g])
                    x16_tiles.append(xt)
            if kp:
                xr = x16pool.tile([128, R16 * F], mybir.dt.float16, tag="xr")
                nc.sync.dma_start(
                    out=xr[:kp, :],
                    in_=x16_d[g16 * G16_ROWS * F :].rearrange(
                        "(k s) -> k s", s=R16 * F
                    ),
                )
            if g8:
                xv8 = x8_d[gp * GP_ROWS * F :].rearrange(
                    "(g k s) -> g k s", k=128, s=R8 * F
                )
                di = 0
                acc_sb = None
                if kd:
                    acc_sb = accpool.tile(
                        [128, kd * F], mybir.dt.float32, tag="acc_sb", name="acc_sb"
                    )
                for g in range(g8):
                    eng = nc.scalar if (TWO_Q and (gp + g) % 2) else nc.sync
                    if g in dve:
                        xt = xdpool.tile([128, R8 * F], FP8_BIR, tag="xd")
                        eng.dma_start(out=xt, in_=xv8[g])
                        wg = w1[:, g * R8 : (g + 1) * R8]
                        tmp = tmppool.tile([128, R8 * F], mybir.dt.float16)
                        # tmp[k, t*F+f] = x[k, t*F+f] * w[k, t]
                        nc.vector.scalar_tensor_tensor(
                            out=tmp.rearrange("k (t f) -> k t f", f=F),
                            in0=xt.rearrange("k (t f) -> k t f", f=F),
                            scalar=1.0,
                            in1=wg.unsqueeze(2).broadcast_to([128, R8, F]),
                            op0=mybir.AluOpType.mult,
                            op1=mybir.AluOpType.mult,
                        )
                        # in-place pairwise tree-sum over slots: every stage
                        # is contiguous stride-1 fp16 (DVE 2x-eligible),
                        # unlike a single strided tensor_reduce (measured
                        # 13.9us vs ~5us for the tree)
                        half = (R8 // 2) * F
                        while half >= F:
                            nc.vector.scalar_tensor_tensor(
                                out=tmp[:, :half],
                                in0=tmp[:, :half],
                                scalar=1.0,
                                in1=tmp[:, half : 2 * half],
                                op0=mybir.AluOpType.mult,
                                op1=mybir.AluOpType.add,
                            )
                            half //= 2
                        nc.vector.tensor_copy(
                            acc_sb[:, di * F : (di + 1) * F], tmp[:, :F]
                        )
                        di += 1
                    else:
                        xt = x8pool.tile([128, R8 * F], FP8_BIR, tag="x8")
                        eng.dma_start(out=xt, in_=xv8[g])
                        mm_unit(
                            lambda j, g=g: w1[
                                :, g * R8 + j * M : g * R8 + (j + 1) * M
                            ],
                            lambda j, xt=xt: xt[:, j * M * F : (j + 1) * M * F],
                            nmm8,
                        )
                if kd:
                    nc.sync.dma_start(out=acc_d[:, :], in_=acc_sb)
            for g, xt in enumerate(x16_tiles):
                mm_unit(
                    lambda j, g=g: w1[
                        :, sl8 + g * R16 + j * M : sl8 + g * R16 + (j + 1) * M
                    ],
                    lambda j, xt=xt: xt[:, j * M * F : (j + 1) * M * F],
                    nmm16,
                )
            if kp:
                off = sl8 + g16 * R16
                mm_unit(
                    lambda j: w1[:kp, off + j * M : off + (j + 1) * M],
                    lambda j: xr[:kp, j * M * F : (j + 1) * M * F],
                    nmm16,
                )
            mm_flush()
            out_sb = opool.tile([32 + M, M * F], mybir.dt.float32)
            nc.vector.tensor_copy(out_sb, psum)
            nc.sync.dma_start(out=out_d[:, :], in_=out_sb)
    nc.compile()
    return nc


def _get_bass(sp: dict) -> bass.Bass:
    key = (
        sp["gp"], sp["g8"], sp["g16"], sp["kp"], sp["dve"],
        R8, XB8, XBD, XB16, TWO_Q, N_WARM,
    )  # fmt: skip
    if key not in _bass_cache:
        _bass_cache[key] = _build_bass(sp)
    return _bass_cache[key]


def _slot_major(wc: np.ndarray, g: int, p: int, r: int) -> np.ndarray:
    """[g*p*r] row-weights -> [128, g*r] slot-major (partition, g*r+t),
    zero-padded from p to 128 partitions."""
    if not g:
        return np.zeros((128, 0), wc.dtype)
    m = np.ascontiguousarray(wc.reshape(g, p, r).transpose(1, 0, 2)).reshape(p, g * r)
    return np.pad(m, ((0, 128 - p), (0, 0))) if p < 128 else m


def _run(x: np.ndarray, w: np.ndarray, trace: bool = False, tmpdir=None):
    """Shard x [n, 64] fp32 + per-row weights w [n] (fp64) over 8 cores,
    return (weighted row-sum [64] as float64, BassKernelResults)."""
    n = x.shape[0]
    sp = _split(n)
    nloc, gp, g8, n8, g16, kp = (
        sp["nloc"], sp["gp"], sp["g8"], sp["n8"], sp["g16"], sp["kp"],
    )  # fmt: skip
    kd = len(sp["dve"])
    n16 = nloc - n8

    in_maps = []
    for c in range(NC):
        lo = c * nloc
        hi = min(lo + nloc, n)
        wc = np.zeros(nloc, np.float16)
        wc[: max(hi - lo, 0)] = w[lo:hi]
        xc8 = np.zeros((n8, F), FP8_NP)
        xc16 = np.zeros((n16, F), np.float16)
        if hi > lo:
            m8 = min(n8, hi - lo)
            xc8[:m8] = x[lo : lo + m8].astype(FP8_NP)
            if hi > lo + n8:
                xc16[: hi - lo - n8] = x[lo + n8 : hi].astype(np.float16)
        npre = gp * GP_ROWS
        wmap = np.concatenate(
            [
                _slot_major(wc[:npre], gp, PREP, R8),
                _slot_major(wc[npre:n8], g8, 128, R8),
                _slot_major(wc[n8 : n8 + g16 * G16_ROWS], g16, 128, R16),
            ]
            + (
                [
                    np.pad(
                        wc[n8 + g16 * G16_ROWS :].reshape(kp, R16),
                        ((0, 128 - kp), (0, 0)),
                    )
                ]
                if kp
                else []
            ),
            axis=1,
        )
        in_maps.append(
            {
                "x8": xc8.reshape(-1) if n8 else np.zeros(F, FP8_NP),
                "x16": xc16.reshape(-1) if n16 else np.zeros(F, np.float16),
                "w": wmap,
            }
        )

    nc = _get_bass(sp)
    res = run_bass_kernel_spmd(
        nc, in_maps, core_ids=list(range(NC)), trace=trace, tmpdir=tmpdir
    )
    n_units = gp + (g8 - kd) + g16 + (1 if kp else 0)
    total = np.zeros(F, np.float64)
    for c in range(NC):
        o = np.asarray(res.results[c]["out"], np.float64)  # [32+M, M*F]
        for t in range(M):
            total += o[t, t * F : (t + 1) * F]
            if n_units >= 2:  # PE column-tile region 1 in psum rows 32:32+M
                total += o[32 + t, t * F : (t + 1) * F]
        if kd:
            a = np.asarray(res.results[c]["acc"], np.float64)  # [128, kd*F]
            total += a.reshape(128, kd, F).sum(axis=(0, 1))
    return total, res


def kernel(x_atom_fea, segment_ids, num_segments=None, **_ignored):
    x = np.asarray(x_atom_fea, dtype=np.float32)
    seg = np.asarray(segment_ids).astype(np.int64, copy=False)
    n0 = int(num_segments) if num_segments is not None else N0_DEFAULT
    counts = np.bincount(seg, minlength=n0)
    # w = 1/count stays in fp16's *normal* range; the 1/N0 factor would
    # push it subnormal and wreck precision, so divide by N0 on the host
    # after the device reduction instead.
    w = 1.0 / np.maximum(counts, 1).astype(np.float64)
    total, _ = _run(x, w[seg])
    return (total / float(n0)).astype(np.float32).reshape(1, F)
